# revision 2
# baseline (speedup 1.0000x reference)
"""Minibatch discrimination kernel v2 — symmetric pair-block schedule, 8 cores.

reference:
    M = (x @ T).reshape(B, K, D)                       # B=1024, K=50, D=5
    abs_diffs[i,k,j] = sum_d |M[i,k,d] - M[j,k,d]|
    feat[i,k] = sum_j exp(-abs_diffs[i,k,j])
    out = concat([x, feat], axis=1)                    # [1024, 562]

v2 exploits |M_i - M_j| symmetry: each unordered block pair {a,b} of the
8x8 (128-row) block grid is computed once. Core m owns query block bi=m
(partitions) vs key blocks bj in {m, m+1, m+2, m+3} (mod 8) for all 50
kernel slots, plus bj=m+4 for slots 0..24 only (its partner core covers
the other 25 via a per-core kernel-slot permutation, keeping the program
SPMD-identical). Row sums of E=exp(-L1) feed feat for bi=m (fused
accum_out); column sums (PE ones-matmuls) feed feat for the bj blocks;
the host adds the partials.

Per-core j axis is rotated by 128*m (host rotates xT per core) so the
owned key blocks are always local columns 0..639. M^T is stored
d-deinterleaved (5 tiles mtd[d][kk, j] = M^T[5*kk+d, j]) so the 5
broadcast matmuls of one kernel slot share a single one-hot lhsT slice
(one weight load per slot instead of five).

Engine split per slot: scalar does Abs planes {0,1}(+{2} on even slots)
fused with the bias subtract + the final Exp (accum_out row-sum); DVE
does plane {2} on odd slots via one tensor_scalar (subtract, abs_max)
plus the 4 fp16 plane adds; gpsimd does planes {3,4} the same one-op way.
"""

import sys

sys.path.insert(0, "/opt/trn_rl_repo")

from contextlib import ExitStack

import numpy as np

import concourse.bass as bass
import concourse.bacc as bacc
import concourse.tile as tile
from concourse import mybir
from concourse.bass_utils import run_bass_kernel_spmd

B, F = 1024, 512
K, D = 50, 5
C = K * D
NCORES = 8
ROWS = B // NCORES  # 128
KH = K // 2  # 25 slots carry the 5th (d4) key block
W_LO, W_HI = 512, 640

f32 = mybir.dt.float32
f16 = mybir.dt.float16

# number of L1-tree adds placed on gpsimd (0 -> all on DVE)
GP_ADDS = 2


def _build_program():
    nc = bacc.Bacc("TRN2", target_bir_lowering=False)

    # xTrot: x.T with j-columns rotated by 128*m (host-prepared per core)
    xTrot = nc.dram_tensor("xTrot", [F, B], f32, kind="ExternalInput").ap()
    # xTloc: core's own 128 query rows of x, transposed (unrotated)
    xTloc = nc.dram_tensor("xTloc", [F, ROWS], f32, kind="ExternalInput").ap()
    # Tm: T with kernel slots permuted per core (slot s = kernel kern_m(s))
    Tm = nc.dram_tensor("Tm", [F, C], f32, kind="ExternalInput").ap()
    onehot = nc.dram_tensor("onehot", [128, 32 * 128], f16, kind="ExternalInput").ap()
    feat_i = nc.dram_tensor("feat_i", [ROWS, K], f32, kind="ExternalOutput").ap()
    # feat_j partials: cols 0:50 -> block m+1, 50:100 -> m+2, 100:150 -> m+3,
    # 150:175 -> block m+4 (slots 0..24)
    feat_j = nc.dram_tensor("feat_j", [ROWS, 175], f32, kind="ExternalOutput").ap()

    sub = mybir.AluOpType.subtract
    add = mybir.AluOpType.add

    with tile.TileContext(nc) as tc, ExitStack() as ctx:
        const_pool = ctx.enter_context(tc.tile_pool(name="const", bufs=1))
        plane_pool = ctx.enter_context(tc.tile_pool(name="planes", bufs=16))
        tmp_pool = ctx.enter_context(tc.tile_pool(name="tmps", bufs=10))
        scratch_pool = ctx.enter_context(tc.tile_pool(name="scratch", bufs=6))

        # ---- load inputs -------------------------------------------------
        xt_sb = []
        t_sb = []
        xtl_sb = []
        for fc in range(4):
            t = const_pool.tile([128, B], f32, tag=f"xt{fc}")
            nc.sync.dma_start(out=t[:], in_=xTrot[128 * fc : 128 * (fc + 1), :])
            xt_sb.append(t)
            t2 = const_pool.tile([128, C], f32, tag=f"tm{fc}")
            nc.sync.dma_start(out=t2[:], in_=Tm[128 * fc : 128 * (fc + 1), :])
            t_sb.append(t2)
            t3 = const_pool.tile([128, ROWS], f32, tag=f"xtl{fc}")
            nc.sync.dma_start(out=t3[:], in_=xTloc[128 * fc : 128 * (fc + 1), :])
            xtl_sb.append(t3)
        oh_sb = const_pool.tile([128, 32 * 128], f16, tag="onehot")
        nc.sync.dma_start(out=oh_sb[:], in_=onehot[:, :])

        # Build-phase PSUM pool is scoped so its banks return to the
        # allocator before the main-loop pools are created.
        with tc.tile_pool(name="bpsum", bufs=2, space="PSUM") as build_psum:
            # One dummy matmul per DMA-queue sem (walrus S3_LW: <=1 sync
            # wait per fused matmul).
            ps_dummy = build_psum.tile([128, 512], f32, tag="bld", name="ps_dummy")
            for dt_tile in xt_sb + t_sb + xtl_sb + [oh_sb]:
                fw = dt_tile.shape[1]
                nc.tensor.matmul(
                    out=ps_dummy[:, 0 : min(512, fw)],
                    lhsT=dt_tile[0:32, 0 : min(128, fw)],
                    rhs=dt_tile[0:32, 0 : min(512, fw)],
                    start=True,
                    stop=True,
                    tile_position=(0, 0),
                )

            # ---- build mtd[d]: [64, 1024] f16, row kk = M^T[5*kk+d, jrot] -
            mtd = []
            for d in range(D):
                t = const_pool.tile([64, B], f16, tag=f"mtd{d}", name=f"mtd{d}")
                nc.vector.memset(t[:, :], 0.0)
                mtd.append(t)
            for d in range(D):
                for jh in range(2):
                    ps = build_psum.tile([128, 512], f32, tag="bld")
                    for fc in range(4):
                        nc.tensor.matmul(
                            out=ps[:K, :],
                            lhsT=t_sb[fc].rearrange("p (k d) -> p d k", d=D)[:, d, :],
                            rhs=xt_sb[fc][:, 512 * jh : 512 * (jh + 1)],
                            start=(fc == 0),
                            stop=(fc == 3),
                        )
                    nc.scalar.copy(mtd[d][:K, 512 * jh : 512 * (jh + 1)], ps[:K, :])

            # ---- build mloc [128, 250] = M for core's own query rows -----
            mloc = const_pool.tile([128, C], f32, tag="mloc")
            ps = build_psum.tile([128, 512], f32, tag="bld")
            for fc in range(4):
                nc.tensor.matmul(
                    out=ps[:, :C],
                    lhsT=xtl_sb[fc][:],
                    rhs=t_sb[fc][:],
                    start=(fc == 0),
                    stop=(fc == 3),
                )
            nc.scalar.copy(mloc[:], ps[:, :C])

        fj_psum = ctx.enter_context(tc.tile_pool(name="fjpsum", bufs=1, space="PSUM"))
        bc_psum = ctx.enter_context(tc.tile_pool(name="bcpsum", bufs=3, space="PSUM"))
        # feat_j accumulator: per (slot, block) column, drained once at end.
        fj_ps = fj_psum.tile([128, 175], f32, tag="fjacc", name="fj_ps")

        feat_sb = const_pool.tile([128, K], f32, tag="feat")
        ones16 = const_pool.tile([128, 1], f16, tag="ones16")
        nc.vector.memset(ones16[:, :], 1.0)

        # ---- main loop, software-pipelined over kernel slots -------------
        # Stage A (slot ks):   PE broadcasts + scalar Abs planes + DVE pa/pb
        # Stage B (slot ks-1): gp t01/t23 + DVE s1/s2/l1
        # Stage C (slot ks-2): scalar Exp (+accum feat_i)
        # Stage D (slot ks-3): PE feat_j column-sum matmuls
        # Delaying each consumer stage keeps every engine's in-order stream
        # free of head-of-line waits on the current slot's chain.
        def wof(ks):
            return W_HI if ks < KH else W_LO

        st = {}  # ks -> dict(planes, pa, pb, l1, ex)

        for ks in range(K + 3):
            if ks < K:
                W = wof(ks)
                bbase = (ks // 32) * 32
                c0 = ks % 32
                planes = []
                for d in range(D):
                    c = 5 * ks + d
                    ps = bc_psum.tile([128, W_HI], f32, tag="bc")
                    nchunks = (W + 511) // 512
                    for jh in range(nchunks):
                        lo = 512 * jh
                        hi = min(W, lo + 512)
                        nc.tensor.matmul(
                            out=ps[:, lo:hi],
                            lhsT=oh_sb[bbase : bbase + 32, 128 * c0 : 128 * (c0 + 1)],
                            rhs=mtd[d][bbase : bbase + 32, lo:hi],
                            start=True,
                            stop=True,
                            tile_position=(bbase, 0),
                        )
                    if d < 4:
                        pl = plane_pool.tile([128, W_HI], f16, tag="plane")
                        nc.scalar.activation(
                            pl[:, :W],
                            ps[:, :W],
                            mybir.ActivationFunctionType.Abs,
                            bias=mloc[:, c : c + 1],
                            scale=-1.0,
                        )
                        planes.append(pl)
                    else:
                        # plane 4 on DVE: pa = relu(x-m), pb = min(x-m, 0);
                        # |x-m| = pa - pb is folded into the L1 tree.
                        pa = plane_pool.tile([128, W_HI], f16, tag="pa")
                        nc.vector.tensor_scalar(
                            pa[:, :W], ps[:, :W], mloc[:, c : c + 1], 0.0,
                            op0=sub, op1=mybir.AluOpType.max,
                        )
                        pb = plane_pool.tile([128, W_HI], f16, tag="pb")
                        nc.vector.tensor_scalar(
                            pb[:, :W], ps[:, :W], mloc[:, c : c + 1], 0.0,
                            op0=sub, op1=mybir.AluOpType.min,
                        )
                st[ks] = {"planes": planes, "pa": pa, "pb": pb}

            kb = ks - 1
            if 0 <= kb < K:
                W = wof(kb)
                s = st[kb]
                planes, pa, pb = s["planes"], s["pa"], s["pb"]
                eng01 = nc.gpsimd if GP_ADDS >= 1 else nc.vector
                t01 = tmp_pool.tile([128, W_HI], f16, tag="t01")
                eng01.tensor_tensor(
                    out=t01[:, :W], in0=planes[0][:, :W], in1=planes[1][:, :W], op=add
                )
                eng23 = nc.gpsimd if GP_ADDS >= 2 else nc.vector
                t23 = tmp_pool.tile([128, W_HI], f16, tag="t23")
                eng23.tensor_tensor(
                    out=t23[:, :W], in0=planes[2][:, :W], in1=planes[3][:, :W], op=add
                )
                s1 = tmp_pool.tile([128, W_HI], f16, tag="s1")
                nc.vector.tensor_tensor(
                    out=s1[:, :W], in0=pa[:, :W], in1=t01[:, :W], op=add
                )
                s2 = tmp_pool.tile([128, W_HI], f16, tag="s2")
                nc.vector.tensor_tensor(
                    out=s2[:, :W], in0=s1[:, :W], in1=t23[:, :W], op=add
                )
                l1 = tmp_pool.tile([128, W_HI], f16, tag="l1")
                nc.vector.tensor_tensor(
                    out=l1[:, :W], in0=s2[:, :W], in1=pb[:, :W], op=sub
                )
                s["l1"] = l1

            kc = ks - 2
            if 0 <= kc < K:
                W = wof(kc)
                s = st[kc]
                ex = scratch_pool.tile([128, W_HI], f16, tag="ex")
                nc.scalar.activation(
                    ex[:, :W],
                    s["l1"][:, :W],
                    mybir.ActivationFunctionType.Exp,
                    bias=0.0,
                    scale=-1.0,
                    accum_out=feat_sb[:, kc : kc + 1],
                )
                s["ex"] = ex

            kd = ks - 3
            if 0 <= kd < K:
                W = wof(kd)
                ex = st[kd]["ex"]
                # column sums for the non-diagonal key blocks -> feat_j psum
                for t in range(1, 4):
                    nc.tensor.matmul(
                        out=fj_ps[:, 50 * (t - 1) + kd : 50 * (t - 1) + kd + 1],
                        lhsT=ex[:, 128 * t : 128 * (t + 1)],
                        rhs=ones16[:, 0:1],
                        start=True,
                        stop=True,
                    )
                if W == W_HI:
                    nc.tensor.matmul(
                        out=fj_ps[:, 150 + kd : 150 + kd + 1],
                        lhsT=ex[:, 512:640],
                        rhs=ones16[:, 0:1],
                        start=True,
                        stop=True,
                    )
                del st[kd]

        fj_sb = const_pool.tile([128, 175], f32, tag="fjsb")
        nc.vector.tensor_scalar(fj_sb[:], fj_ps[:], 0.0, None, op0=add)
        nc.sync.dma_start(out=feat_i[:, :], in_=feat_sb[:, :K])
        nc.sync.dma_start(out=feat_j[:, :], in_=fj_sb[:, :])

    nc.compile()
    return nc


_program_cache = {}


def _get_program():
    if "nc" not in _program_cache:
        _program_cache["nc"] = _build_program()
    return _program_cache["nc"]


def _make_onehot():
    oh = np.zeros((128, 32 * 128), dtype=np.float16)
    for p in range(128):
        oh[p, (p % 32) * 128 : (p % 32 + 1) * 128] = 1.0
    return oh


def kernel(x: np.ndarray, T: np.ndarray, _trace=False, _trace_kwargs=None):
    x = np.asarray(x, dtype=np.float32)
    T = np.asarray(T, dtype=np.float32)
    nc = _get_program()

    xT_full = np.ascontiguousarray(x.T)  # [512, 1024]
    oh = _make_onehot()
    T5 = T.reshape(F, K, D)
    in_maps = []
    kmaps = []
    for m in range(NCORES):
        # kernel-slot permutation: slots 0..24 must be this core's d4 half
        if m < 4:
            kmap = np.arange(K)
        else:
            kmap = np.concatenate([np.arange(KH, K), np.arange(0, KH)])
        kmaps.append(kmap)
        Tperm = np.ascontiguousarray(T5[:, kmap, :].reshape(F, C))
        in_maps.append(
            {
                "xTrot": np.ascontiguousarray(np.roll(xT_full, -ROWS * m, axis=1)),
                "xTloc": np.ascontiguousarray(xT_full[:, ROWS * m : ROWS * (m + 1)]),
                "Tm": Tperm,
                "onehot": oh,
            }
        )

    res = run_bass_kernel_spmd(
        nc,
        in_maps,
        core_ids=list(range(NCORES)),
        trace=_trace,
        **(_trace_kwargs or {}),
    )

    feats = np.zeros((B, K), dtype=np.float32)
    for m in range(NCORES):
        kmap = kmaps[m]
        fi = np.asarray(res.results[m]["feat_i"], dtype=np.float32)  # [128, 50]
        fj = np.asarray(res.results[m]["feat_j"], dtype=np.float32)  # [128, 175]
        rows_i = slice(ROWS * m, ROWS * (m + 1))
        # feat_i: row sums over owned key blocks, slot s = kernel kmap[s]
        feats[rows_i][:, kmap] += fi
        # feat_j partials for blocks m+1..m+3 (all slots) and m+4 (slots<25)
        for t in range(1, 4):
            rows_j = np.arange(ROWS * ((m + t) % 8), ROWS * ((m + t) % 8) + ROWS)
            feats[rows_j[:, None], kmap[None, :]] += fj[:, 50 * (t - 1) : 50 * t]
        rows_j = np.arange(ROWS * ((m + 4) % 8), ROWS * ((m + 4) % 8) + ROWS)
        feats[rows_j[:, None], kmap[None, :KH]] += fj[:, 150:175]

    out = np.concatenate([x, feats], axis=1)
    if _trace:
        return out, res
    return out


# revision 3
# speedup vs baseline: 1.0337x; 1.0337x over previous
"""Minibatch discrimination kernel v2 — symmetric pair-block schedule, 8 cores.

reference:
    M = (x @ T).reshape(B, K, D)                       # B=1024, K=50, D=5
    abs_diffs[i,k,j] = sum_d |M[i,k,d] - M[j,k,d]|
    feat[i,k] = sum_j exp(-abs_diffs[i,k,j])
    out = concat([x, feat], axis=1)                    # [1024, 562]

v2 exploits |M_i - M_j| symmetry: each unordered block pair {a,b} of the
8x8 (128-row) block grid is computed once. Core m owns query block bi=m
(partitions) vs key blocks bj in {m, m+1, m+2, m+3} (mod 8) for all 50
kernel slots, plus bj=m+4 for slots 0..24 only (its partner core covers
the other 25 via a per-core kernel-slot permutation, keeping the program
SPMD-identical). Row sums of E=exp(-L1) feed feat for bi=m (fused
accum_out); column sums (PE ones-matmuls) feed feat for the bj blocks;
the host adds the partials.

Per-core j axis is rotated by 128*m (host rotates xT per core) so the
owned key blocks are always local columns 0..639. M^T is stored
d-deinterleaved (5 tiles mtd[d][kk, j] = M^T[5*kk+d, j]) so the 5
broadcast matmuls of one kernel slot share a single one-hot lhsT slice
(one weight load per slot instead of five).

Engine split per slot: scalar does Abs planes {0,1}(+{2} on even slots)
fused with the bias subtract + the final Exp (accum_out row-sum); DVE
does plane {2} on odd slots via one tensor_scalar (subtract, abs_max)
plus the 4 fp16 plane adds; gpsimd does planes {3,4} the same one-op way.
"""

import sys

sys.path.insert(0, "/opt/trn_rl_repo")

from contextlib import ExitStack

import numpy as np

import concourse.bass as bass
import concourse.bacc as bacc
import concourse.tile as tile
from concourse import mybir
from concourse.bass_utils import run_bass_kernel_spmd

B, F = 1024, 512
K, D = 50, 5
C = K * D
NCORES = 8
ROWS = B // NCORES  # 128
KH = K // 2  # 25 slots carry the 5th (d4) key block
W_LO, W_HI = 512, 640

f32 = mybir.dt.float32
f16 = mybir.dt.float16

# number of L1-tree adds placed on gpsimd (0 -> all on DVE)
GP_ADDS = 2


def _build_program():
    nc = bacc.Bacc("TRN2", target_bir_lowering=False)

    # xTrot: x.T with j-columns rotated by 128*m (host-prepared per core)
    xTrot = nc.dram_tensor("xTrot", [F, B], f32, kind="ExternalInput").ap()
    # xTloc: core's own 128 query rows of x, transposed (unrotated)
    xTloc = nc.dram_tensor("xTloc", [F, ROWS], f32, kind="ExternalInput").ap()
    # Tm: T with kernel slots permuted per core (slot s = kernel kern_m(s))
    Tm = nc.dram_tensor("Tm", [F, C], f32, kind="ExternalInput").ap()
    onehot = nc.dram_tensor("onehot", [128, 32 * 128], f16, kind="ExternalInput").ap()
    feat_i = nc.dram_tensor("feat_i", [ROWS, K], f32, kind="ExternalOutput").ap()
    # feat_j partials: cols 0:50 -> block m+1, 50:100 -> m+2, 100:150 -> m+3,
    # 150:175 -> block m+4 (slots 0..24)
    feat_j = nc.dram_tensor("feat_j", [ROWS, 175], f32, kind="ExternalOutput").ap()

    sub = mybir.AluOpType.subtract
    add = mybir.AluOpType.add

    with tile.TileContext(nc) as tc, ExitStack() as ctx:
        const_pool = ctx.enter_context(tc.tile_pool(name="const", bufs=1))
        plane_pool = ctx.enter_context(tc.tile_pool(name="planes", bufs=16))
        tmp_pool = ctx.enter_context(tc.tile_pool(name="tmps", bufs=10))
        scratch_pool = ctx.enter_context(tc.tile_pool(name="scratch", bufs=6))

        # ---- load inputs -------------------------------------------------
        xt_sb = []
        t_sb = []
        xtl_sb = []
        for fc in range(4):
            t = const_pool.tile([128, B], f32, tag=f"xt{fc}")
            nc.sync.dma_start(out=t[:], in_=xTrot[128 * fc : 128 * (fc + 1), :])
            xt_sb.append(t)
            t2 = const_pool.tile([128, C], f32, tag=f"tm{fc}")
            nc.sync.dma_start(out=t2[:], in_=Tm[128 * fc : 128 * (fc + 1), :])
            t_sb.append(t2)
            t3 = const_pool.tile([128, ROWS], f32, tag=f"xtl{fc}")
            nc.sync.dma_start(out=t3[:], in_=xTloc[128 * fc : 128 * (fc + 1), :])
            xtl_sb.append(t3)
        oh_sb = const_pool.tile([128, 32 * 128], f16, tag="onehot")
        nc.sync.dma_start(out=oh_sb[:], in_=onehot[:, :])

        # Build-phase PSUM pool is scoped so its banks return to the
        # allocator before the main-loop pools are created.
        with tc.tile_pool(name="bpsum", bufs=2, space="PSUM") as build_psum:
            # One dummy matmul per DMA-queue sem (walrus S3_LW: <=1 sync
            # wait per fused matmul).
            ps_dummy = build_psum.tile([128, 512], f32, tag="bld", name="ps_dummy")
            for dt_tile in xt_sb + t_sb + xtl_sb + [oh_sb]:
                fw = dt_tile.shape[1]
                nc.tensor.matmul(
                    out=ps_dummy[:, 0 : min(512, fw)],
                    lhsT=dt_tile[0:32, 0 : min(128, fw)],
                    rhs=dt_tile[0:32, 0 : min(512, fw)],
                    start=True,
                    stop=True,
                    tile_position=(0, 0),
                )

            # ---- build mtd[d]: [64, 1024] f16, row kk = M^T[5*kk+d, jrot] -
            mtd = []
            for d in range(D):
                t = const_pool.tile([64, B], f16, tag=f"mtd{d}", name=f"mtd{d}")
                nc.vector.memset(t[:, :], 0.0)
                mtd.append(t)
            for d in range(D):
                for jh in range(2):
                    ps = build_psum.tile([128, 512], f32, tag="bld")
                    for fc in range(4):
                        nc.tensor.matmul(
                            out=ps[:K, :],
                            lhsT=t_sb[fc].rearrange("p (k d) -> p d k", d=D)[:, d, :],
                            rhs=xt_sb[fc][:, 512 * jh : 512 * (jh + 1)],
                            start=(fc == 0),
                            stop=(fc == 3),
                        )
                    nc.scalar.copy(mtd[d][:K, 512 * jh : 512 * (jh + 1)], ps[:K, :])

            # ---- build mloc [128, 250] = M for core's own query rows -----
            mloc = const_pool.tile([128, C], f32, tag="mloc")
            ps = build_psum.tile([128, 512], f32, tag="bld")
            for fc in range(4):
                nc.tensor.matmul(
                    out=ps[:, :C],
                    lhsT=xtl_sb[fc][:],
                    rhs=t_sb[fc][:],
                    start=(fc == 0),
                    stop=(fc == 3),
                )
            nc.scalar.copy(mloc[:], ps[:, :C])

        fj_psum = ctx.enter_context(tc.tile_pool(name="fjpsum", bufs=1, space="PSUM"))
        bc_psum = ctx.enter_context(tc.tile_pool(name="bcpsum", bufs=3, space="PSUM"))
        # feat_j accumulator: per (slot, block) column, drained once at end.
        fj_ps = fj_psum.tile([128, 175], f32, tag="fjacc", name="fj_ps")

        feat_sb = const_pool.tile([128, K], f32, tag="feat")
        ones16 = const_pool.tile([128, 1], f16, tag="ones16")
        nc.vector.memset(ones16[:, :], 1.0)

        # ---- main loop, software-pipelined over kernel slots -------------
        # Stage A (slot ks):   PE broadcasts + scalar Abs planes + DVE pa/pb
        # Stage B (slot ks-1): gp t01/t23 + DVE s1/s2/l1
        # Stage C (slot ks-2): scalar Exp (+accum feat_i)
        # Stage D (slot ks-3): PE feat_j column-sum matmuls
        # Delaying each consumer stage keeps every engine's in-order stream
        # free of head-of-line waits on the current slot's chain.
        def wof(ks):
            return W_HI if ks < KH else W_LO

        st = {}  # ks -> dict(planes, pa, pb, l1, ex)

        for ks in range(K + 3):
            if ks < K:
                W = wof(ks)
                bbase = (ks // 32) * 32
                c0 = ks % 32
                planes = []
                for d in range(D):
                    c = 5 * ks + d
                    ps = bc_psum.tile([128, W_HI], f32, tag="bc")
                    nchunks = (W + 511) // 512
                    for jh in range(nchunks):
                        lo = 512 * jh
                        hi = min(W, lo + 512)
                        nc.tensor.matmul(
                            out=ps[:, lo:hi],
                            lhsT=oh_sb[bbase : bbase + 32, 128 * c0 : 128 * (c0 + 1)],
                            rhs=mtd[d][bbase : bbase + 32, lo:hi],
                            start=True,
                            stop=True,
                            tile_position=(bbase, 0),
                        )
                    if d < 4:
                        pl = plane_pool.tile([128, W_HI], f16, tag="plane")
                        nc.scalar.activation(
                            pl[:, :W],
                            ps[:, :W],
                            mybir.ActivationFunctionType.Abs,
                            bias=mloc[:, c : c + 1],
                            scale=-1.0,
                        )
                        planes.append(pl)
                    else:
                        # plane 4 on DVE. Drain PSUM with ONE read (df =
                        # x - m, fp16 SBUF) so the bank frees immediately —
                        # the next slot's broadcasts no longer wait on a
                        # two-read drain. pa/pb then run from SBUF at the
                        # fast DVE mode; |x-m| = pa - pb folds into the tree.
                        df = plane_pool.tile([128, W_HI], f16, tag="df")
                        nc.vector.tensor_scalar(
                            df[:, :W], ps[:, :W], mloc[:, c : c + 1], None,
                            op0=sub,
                        )
                        pa = plane_pool.tile([128, W_HI], f16, tag="pa")
                        nc.vector.tensor_scalar(
                            pa[:, :W], df[:, :W], 0.0, None,
                            op0=mybir.AluOpType.max,
                        )
                        pb = plane_pool.tile([128, W_HI], f16, tag="pb")
                        nc.vector.tensor_scalar(
                            pb[:, :W], df[:, :W], 0.0, None,
                            op0=mybir.AluOpType.min,
                        )
                st[ks] = {"planes": planes, "pa": pa, "pb": pb}

            kb = ks - 1
            if 0 <= kb < K:
                W = wof(kb)
                s = st[kb]
                planes, pa, pb = s["planes"], s["pa"], s["pb"]
                eng01 = nc.gpsimd if GP_ADDS >= 1 else nc.vector
                t01 = tmp_pool.tile([128, W_HI], f16, tag="t01")
                eng01.tensor_tensor(
                    out=t01[:, :W], in0=planes[0][:, :W], in1=planes[1][:, :W], op=add
                )
                eng23 = nc.gpsimd if GP_ADDS >= 2 else nc.vector
                t23 = tmp_pool.tile([128, W_HI], f16, tag="t23")
                eng23.tensor_tensor(
                    out=t23[:, :W], in0=planes[2][:, :W], in1=planes[3][:, :W], op=add
                )
                s1 = tmp_pool.tile([128, W_HI], f16, tag="s1")
                nc.vector.tensor_tensor(
                    out=s1[:, :W], in0=pa[:, :W], in1=t01[:, :W], op=add
                )
                s2 = tmp_pool.tile([128, W_HI], f16, tag="s2")
                nc.vector.tensor_tensor(
                    out=s2[:, :W], in0=s1[:, :W], in1=t23[:, :W], op=add
                )
                l1 = tmp_pool.tile([128, W_HI], f16, tag="l1")
                nc.vector.tensor_tensor(
                    out=l1[:, :W], in0=s2[:, :W], in1=pb[:, :W], op=sub
                )
                s["l1"] = l1

            kc = ks - 2
            if 0 <= kc < K:
                W = wof(kc)
                s = st[kc]
                ex = scratch_pool.tile([128, W_HI], f16, tag="ex")
                nc.scalar.activation(
                    ex[:, :W],
                    s["l1"][:, :W],
                    mybir.ActivationFunctionType.Exp,
                    bias=0.0,
                    scale=-1.0,
                    accum_out=feat_sb[:, kc : kc + 1],
                )
                s["ex"] = ex

            kd = ks - 3
            if 0 <= kd < K:
                W = wof(kd)
                ex = st[kd]["ex"]
                # column sums for the non-diagonal key blocks -> feat_j psum
                for t in range(1, 4):
                    nc.tensor.matmul(
                        out=fj_ps[:, 50 * (t - 1) + kd : 50 * (t - 1) + kd + 1],
                        lhsT=ex[:, 128 * t : 128 * (t + 1)],
                        rhs=ones16[:, 0:1],
                        start=True,
                        stop=True,
                    )
                if W == W_HI:
                    nc.tensor.matmul(
                        out=fj_ps[:, 150 + kd : 150 + kd + 1],
                        lhsT=ex[:, 512:640],
                        rhs=ones16[:, 0:1],
                        start=True,
                        stop=True,
                    )
                del st[kd]

        fj_sb = const_pool.tile([128, 175], f32, tag="fjsb")
        nc.vector.tensor_scalar(fj_sb[:], fj_ps[:], 0.0, None, op0=add)
        nc.sync.dma_start(out=feat_i[:, :], in_=feat_sb[:, :K])
        nc.sync.dma_start(out=feat_j[:, :], in_=fj_sb[:, :])

    nc.compile()
    return nc


_program_cache = {}


def _get_program():
    if "nc" not in _program_cache:
        _program_cache["nc"] = _build_program()
    return _program_cache["nc"]


def _make_onehot():
    oh = np.zeros((128, 32 * 128), dtype=np.float16)
    for p in range(128):
        oh[p, (p % 32) * 128 : (p % 32 + 1) * 128] = 1.0
    return oh


def kernel(x: np.ndarray, T: np.ndarray, _trace=False, _trace_kwargs=None):
    x = np.asarray(x, dtype=np.float32)
    T = np.asarray(T, dtype=np.float32)
    nc = _get_program()

    xT_full = np.ascontiguousarray(x.T)  # [512, 1024]
    oh = _make_onehot()
    T5 = T.reshape(F, K, D)
    in_maps = []
    kmaps = []
    for m in range(NCORES):
        # kernel-slot permutation: slots 0..24 must be this core's d4 half
        if m < 4:
            kmap = np.arange(K)
        else:
            kmap = np.concatenate([np.arange(KH, K), np.arange(0, KH)])
        kmaps.append(kmap)
        Tperm = np.ascontiguousarray(T5[:, kmap, :].reshape(F, C))
        in_maps.append(
            {
                "xTrot": np.ascontiguousarray(np.roll(xT_full, -ROWS * m, axis=1)),
                "xTloc": np.ascontiguousarray(xT_full[:, ROWS * m : ROWS * (m + 1)]),
                "Tm": Tperm,
                "onehot": oh,
            }
        )

    res = run_bass_kernel_spmd(
        nc,
        in_maps,
        core_ids=list(range(NCORES)),
        trace=_trace,
        **(_trace_kwargs or {}),
    )

    feats = np.zeros((B, K), dtype=np.float32)
    for m in range(NCORES):
        kmap = kmaps[m]
        fi = np.asarray(res.results[m]["feat_i"], dtype=np.float32)  # [128, 50]
        fj = np.asarray(res.results[m]["feat_j"], dtype=np.float32)  # [128, 175]
        rows_i = slice(ROWS * m, ROWS * (m + 1))
        # feat_i: row sums over owned key blocks, slot s = kernel kmap[s]
        feats[rows_i][:, kmap] += fi
        # feat_j partials for blocks m+1..m+3 (all slots) and m+4 (slots<25)
        for t in range(1, 4):
            rows_j = np.arange(ROWS * ((m + t) % 8), ROWS * ((m + t) % 8) + ROWS)
            feats[rows_j[:, None], kmap[None, :]] += fj[:, 50 * (t - 1) : 50 * t]
        rows_j = np.arange(ROWS * ((m + 4) % 8), ROWS * ((m + 4) % 8) + ROWS)
        feats[rows_j[:, None], kmap[None, :KH]] += fj[:, 150:175]

    out = np.concatenate([x, feats], axis=1)
    if _trace:
        return out, res
    return out


# revision 10
# speedup vs baseline: 1.2380x; 1.1976x over previous
"""Minibatch discrimination kernel v2 — symmetric pair-block schedule, 8 cores.

reference:
    M = (x @ T).reshape(B, K, D)                       # B=1024, K=50, D=5
    abs_diffs[i,k,j] = sum_d |M[i,k,d] - M[j,k,d]|
    feat[i,k] = sum_j exp(-abs_diffs[i,k,j])
    out = concat([x, feat], axis=1)                    # [1024, 562]

v2 exploits |M_i - M_j| symmetry: each unordered block pair {a,b} of the
8x8 (128-row) block grid is computed once. Core m owns query block bi=m
(partitions) vs key blocks bj in {m, m+1, m+2, m+3} (mod 8) for all 50
kernel slots, plus bj=m+4 for slots 0..24 only (its partner core covers
the other 25 via a per-core kernel-slot permutation, keeping the program
SPMD-identical). Row sums of E=exp(-L1) feed feat for bi=m (fused
accum_out); column sums (PE ones-matmuls) feed feat for the bj blocks;
the host adds the partials.

Per-core j axis is rotated by 128*m (host rotates xT per core) so the
owned key blocks are always local columns 0..639. M^T is stored
d-deinterleaved (5 tiles mtd[d][kk, j] = M^T[5*kk+d, j]) so the 5
broadcast matmuls of one kernel slot share a single one-hot lhsT slice
(one weight load per slot instead of five).

Engine split per slot: scalar does Abs planes {0,1}(+{2} on even slots)
fused with the bias subtract + the final Exp (accum_out row-sum); DVE
does plane {2} on odd slots via one tensor_scalar (subtract, abs_max)
plus the 4 fp16 plane adds; gpsimd does planes {3,4} the same one-op way.
"""

import sys

sys.path.insert(0, "/opt/trn_rl_repo")

from contextlib import ExitStack

import numpy as np

import concourse.bass as bass
import concourse.bacc as bacc
import concourse.tile as tile
from concourse import mybir
from concourse.bass_utils import run_bass_kernel_spmd

B, F = 1024, 512
K, D = 50, 5
C = K * D
NCORES = 8
ROWS = B // NCORES  # 128
KH = K // 2  # 25 slots carry the 5th (d4) key block
W_LO, W_HI = 512, 640

f32 = mybir.dt.float32
f16 = mybir.dt.float16

# number of L1-tree adds placed on gpsimd (0 -> all on DVE)
GP_ADDS = 2


def _build_program():
    nc = bacc.Bacc("TRN2", target_bir_lowering=False)

    # Inputs are fp16 (host-converted): halves the load DMAs and runs the
    # M build matmuls at 1 cycle/row instead of fp32's 4.
    # xTrot: x.T with j-columns rotated by 128*m (host-prepared per core)
    xTrot = nc.dram_tensor("xTrot", [F, B], f16, kind="ExternalInput").ap()
    # xTloc: core's own 128 query rows of x, transposed (unrotated)
    xTloc = nc.dram_tensor("xTloc", [F, ROWS], f16, kind="ExternalInput").ap()
    # Tm: T with kernel slots permuted per core (slot s = kernel kern_m(s))
    Tm = nc.dram_tensor("Tm", [F, C], f16, kind="ExternalInput").ap()
    onehot = nc.dram_tensor("onehot", [64, 32 * 128], f16, kind="ExternalInput").ap()
    feat_i = nc.dram_tensor("feat_i", [ROWS, K], f32, kind="ExternalOutput").ap()
    # feat_j partials: cols 0:50 -> block m+1, 50:100 -> m+2, 100:150 -> m+3,
    # 150:175 -> block m+4 (slots 0..24)
    feat_j = nc.dram_tensor("feat_j", [ROWS, 175], f32, kind="ExternalOutput").ap()

    sub = mybir.AluOpType.subtract
    add = mybir.AluOpType.add

    with tile.TileContext(nc) as tc, ExitStack() as ctx:
        const_pool = ctx.enter_context(tc.tile_pool(name="const", bufs=1))
        plane_pool = ctx.enter_context(tc.tile_pool(name="planes", bufs=16))
        tmp_pool = ctx.enter_context(tc.tile_pool(name="tmps", bufs=10))
        scratch_pool = ctx.enter_context(tc.tile_pool(name="scratch", bufs=6))

        # ---- load inputs -------------------------------------------------
        xt_sb = []
        t_sb = []
        xtl_sb = []
        # Input loads round-robin across engine DMA queues — a single
        # queue moves ~26GB/s and serializes the whole setup phase.
        qs = [nc.sync, nc.scalar, nc.gpsimd]
        for fc in range(4):
            t = const_pool.tile([128, B], f16, tag=f"xt{fc}")
            qs[fc % 3].dma_start(out=t[:], in_=xTrot[128 * fc : 128 * (fc + 1), :])
            xt_sb.append(t)
            t2 = const_pool.tile([128, C], f16, tag=f"tm{fc}")
            qs[(fc + 1) % 3].dma_start(out=t2[:], in_=Tm[128 * fc : 128 * (fc + 1), :])
            t_sb.append(t2)
            t3 = const_pool.tile([128, ROWS], f16, tag=f"xtl{fc}")
            qs[(fc + 2) % 3].dma_start(
                out=t3[:], in_=xTloc[128 * fc : 128 * (fc + 1), :]
            )
            xtl_sb.append(t3)
        oh_sb = const_pool.tile([64, 32 * 128], f16, tag="onehot")
        nc.scalar.dma_start(out=oh_sb[:], in_=onehot[:, :])

        # Build-phase PSUM pool is scoped so its banks return to the
        # allocator before the main-loop pools are created.
        with tc.tile_pool(name="bpsum", bufs=2, space="PSUM") as build_psum:
            # One dummy matmul per DMA-queue sem (walrus S3_LW: <=1 sync
            # wait per fused matmul).
            ps_dummy = build_psum.tile([128, 512], f32, tag="bld", name="ps_dummy")
            for dt_tile in xt_sb + t_sb + xtl_sb + [oh_sb]:
                fw = dt_tile.shape[1]
                nc.tensor.matmul(
                    out=ps_dummy[:, 0 : min(512, fw)],
                    lhsT=dt_tile[0:32, 0 : min(128, fw)],
                    rhs=dt_tile[0:32, 0 : min(512, fw)],
                    start=True,
                    stop=True,
                    tile_position=(0, 0),
                )

            # ---- build mtd[d]: [64, 1024] f16, row kk = M^T[5*kk+d, jrot] -
            mtd = []
            for d in range(D):
                t = const_pool.tile([64, B], f16, tag=f"mtd{d}", name=f"mtd{d}")
                nc.vector.memset(t[:, :], 0.0)
                mtd.append(t)
            for d in range(D):
                for jh in range(2):
                    ps = build_psum.tile([128, 512], f32, tag="bld")
                    for fc in range(4):
                        nc.tensor.matmul(
                            out=ps[:K, :],
                            lhsT=t_sb[fc].rearrange("p (k d) -> p d k", d=D)[:, d, :],
                            rhs=xt_sb[fc][:, 512 * jh : 512 * (jh + 1)],
                            start=(fc == 0),
                            stop=(fc == 3),
                        )
                    nc.scalar.copy(mtd[d][:K, 512 * jh : 512 * (jh + 1)], ps[:K, :])

            # ---- build mloc [128, 250] = M for core's own query rows -----
            mloc = const_pool.tile([128, C], f32, tag="mloc")
            ps = build_psum.tile([128, 512], f32, tag="bld")
            for fc in range(4):
                nc.tensor.matmul(
                    out=ps[:, :C],
                    lhsT=xtl_sb[fc][:],
                    rhs=t_sb[fc][:],
                    start=(fc == 0),
                    stop=(fc == 3),
                )
            nc.scalar.copy(mloc[:], ps[:, :C])

        fj_psum = ctx.enter_context(tc.tile_pool(name="fjpsum", bufs=1, space="PSUM"))
        bc_psum = ctx.enter_context(tc.tile_pool(name="bcpsum", bufs=3, space="PSUM"))
        # feat_j accumulator: per (slot, block) column, drained once at end.
        fj_ps = fj_psum.tile([128, 175], f32, tag="fjacc", name="fj_ps")

        feat_sb = const_pool.tile([128, K], f32, tag="feat")
        ones16 = const_pool.tile([128, 1], f16, tag="ones16")
        nc.vector.memset(ones16[:, :], 1.0)

        # ---- main loop, software-pipelined over kernel slots -------------
        # Stage A (slot ks):   PE broadcasts + scalar Abs planes + DVE pa/pb
        # Stage B (slot ks-1): gp t01/t23 + DVE s1/s2/l1
        # Stage C (slot ks-2): scalar Exp (+accum feat_i)
        # Stage D (slot ks-3): PE feat_j column-sum matmuls
        # Delaying each consumer stage keeps every engine's in-order stream
        # free of head-of-line waits on the current slot's chain.
        def wof(ks):
            return W_HI if ks < KH else W_LO

        st = {}  # ks -> dict(planes, pa, pb, l1, ex)

        for ks in range(K + 3):
            if ks < K:
                W = wof(ks)
                bbase = (ks // 32) * 32
                c0 = ks % 32
                planes = []
                for d in range(D):
                    c = 5 * ks + d
                    ps = bc_psum.tile([128, W_HI], f32, tag="bc")
                    nchunks = (W + 511) // 512
                    for jh in range(nchunks):
                        lo = 512 * jh
                        hi = min(W, lo + 512)
                        nc.tensor.matmul(
                            out=ps[:, lo:hi],
                            lhsT=oh_sb[bbase : bbase + 32, 128 * c0 : 128 * (c0 + 1)],
                            rhs=mtd[d][bbase : bbase + 32, lo:hi],
                            start=True,
                            stop=True,
                            tile_position=(bbase, 0),
                        )
                    if d < 4:
                        pl = plane_pool.tile([128, W_HI], f16, tag="plane")
                        nc.scalar.activation(
                            pl[:, :W],
                            ps[:, :W],
                            mybir.ActivationFunctionType.Abs,
                            bias=mloc[:, c : c + 1],
                            scale=-1.0,
                        )
                        planes.append(pl)
                    else:
                        # plane 4 on DVE: pa = relu(x-m), pb = min(x-m, 0);
                        # |x-m| = pa - pb is folded into the L1 tree.
                        pa = plane_pool.tile([128, W_HI], f16, tag="pa")
                        nc.vector.tensor_scalar(
                            pa[:, :W], ps[:, :W], mloc[:, c : c + 1], 0.0,
                            op0=sub, op1=mybir.AluOpType.max,
                        )
                        pb = plane_pool.tile([128, W_HI], f16, tag="pb")
                        nc.vector.tensor_scalar(
                            pb[:, :W], ps[:, :W], mloc[:, c : c + 1], 0.0,
                            op0=sub, op1=mybir.AluOpType.min,
                        )
                st[ks] = {"planes": planes, "pa": pa, "pb": pb}

            kb = ks - 1
            if 0 <= kb < K:
                W = wof(kb)
                s = st[kb]
                planes, pa, pb = s["planes"], s["pa"], s["pb"]
                eng01 = nc.gpsimd if GP_ADDS >= 1 else nc.vector
                t01 = tmp_pool.tile([128, W_HI], f16, tag="t01")
                eng01.tensor_tensor(
                    out=t01[:, :W], in0=planes[0][:, :W], in1=planes[1][:, :W], op=add
                )
                eng23 = nc.gpsimd if GP_ADDS >= 2 else nc.vector
                t23 = tmp_pool.tile([128, W_HI], f16, tag="t23")
                eng23.tensor_tensor(
                    out=t23[:, :W], in0=planes[2][:, :W], in1=planes[3][:, :W], op=add
                )
                s1 = tmp_pool.tile([128, W_HI], f16, tag="s1")
                nc.vector.tensor_tensor(
                    out=s1[:, :W], in0=pa[:, :W], in1=t01[:, :W], op=add
                )
                s2 = tmp_pool.tile([128, W_HI], f16, tag="s2")
                nc.vector.tensor_tensor(
                    out=s2[:, :W], in0=s1[:, :W], in1=t23[:, :W], op=add
                )
                l1 = tmp_pool.tile([128, W_HI], f16, tag="l1")
                nc.vector.tensor_tensor(
                    out=l1[:, :W], in0=s2[:, :W], in1=pb[:, :W], op=sub
                )
                s["l1"] = l1

            kc = ks - 2
            if 0 <= kc < K:
                W = wof(kc)
                s = st[kc]
                ex = scratch_pool.tile([128, W_HI], f16, tag="ex")
                nc.scalar.activation(
                    ex[:, :W],
                    s["l1"][:, :W],
                    mybir.ActivationFunctionType.Exp,
                    bias=0.0,
                    scale=-1.0,
                    accum_out=feat_sb[:, kc : kc + 1],
                )
                s["ex"] = ex

            kd = ks - 3
            if 0 <= kd < K:
                W = wof(kd)
                ex = st[kd]["ex"]
                # column sums for the non-diagonal key blocks -> feat_j psum
                for t in range(1, 4):
                    nc.tensor.matmul(
                        out=fj_ps[:, 50 * (t - 1) + kd : 50 * (t - 1) + kd + 1],
                        lhsT=ex[:, 128 * t : 128 * (t + 1)],
                        rhs=ones16[:, 0:1],
                        start=True,
                        stop=True,
                    )
                if W == W_HI:
                    nc.tensor.matmul(
                        out=fj_ps[:, 150 + kd : 150 + kd + 1],
                        lhsT=ex[:, 512:640],
                        rhs=ones16[:, 0:1],
                        start=True,
                        stop=True,
                    )
                del st[kd]

        fj_sb = const_pool.tile([128, 175], f32, tag="fjsb")
        nc.vector.tensor_scalar(fj_sb[:], fj_ps[:], 0.0, None, op0=add)
        nc.sync.dma_start(out=feat_i[:, :], in_=feat_sb[:, :K])
        nc.sync.dma_start(out=feat_j[:, :], in_=fj_sb[:, :])

    nc.compile()
    return nc


_program_cache = {}


def _get_program():
    if "nc" not in _program_cache:
        _program_cache["nc"] = _build_program()
    return _program_cache["nc"]


def _make_onehot():
    oh = np.zeros((64, 32 * 128), dtype=np.float16)
    for p in range(64):
        oh[p, (p % 32) * 128 : (p % 32 + 1) * 128] = 1.0
    return oh


def kernel(x: np.ndarray, T: np.ndarray, _trace=False, _trace_kwargs=None):
    x = np.asarray(x, dtype=np.float32)
    T = np.asarray(T, dtype=np.float32)
    nc = _get_program()

    xT_full = np.ascontiguousarray(x.T)  # [512, 1024]
    oh = _make_onehot()
    T5 = T.reshape(F, K, D)
    in_maps = []
    kmaps = []
    for m in range(NCORES):
        # kernel-slot permutation: slots 0..24 must be this core's d4 half
        if m < 4:
            kmap = np.arange(K)
        else:
            kmap = np.concatenate([np.arange(KH, K), np.arange(0, KH)])
        kmaps.append(kmap)
        Tperm = np.ascontiguousarray(T5[:, kmap, :].reshape(F, C)).astype(np.float16)
        in_maps.append(
            {
                "xTrot": np.ascontiguousarray(
                    np.roll(xT_full, -ROWS * m, axis=1)
                ).astype(np.float16),
                "xTloc": np.ascontiguousarray(
                    xT_full[:, ROWS * m : ROWS * (m + 1)]
                ).astype(np.float16),
                "Tm": Tperm,
                "onehot": oh,
            }
        )

    res = run_bass_kernel_spmd(
        nc,
        in_maps,
        core_ids=list(range(NCORES)),
        trace=_trace,
        **(_trace_kwargs or {}),
    )

    feats = np.zeros((B, K), dtype=np.float32)
    for m in range(NCORES):
        kmap = kmaps[m]
        fi = np.asarray(res.results[m]["feat_i"], dtype=np.float32)  # [128, 50]
        fj = np.asarray(res.results[m]["feat_j"], dtype=np.float32)  # [128, 175]
        rows_i = slice(ROWS * m, ROWS * (m + 1))
        # feat_i: row sums over owned key blocks, slot s = kernel kmap[s]
        feats[rows_i][:, kmap] += fi
        # feat_j partials for blocks m+1..m+3 (all slots) and m+4 (slots<25)
        for t in range(1, 4):
            rows_j = np.arange(ROWS * ((m + t) % 8), ROWS * ((m + t) % 8) + ROWS)
            feats[rows_j[:, None], kmap[None, :]] += fj[:, 50 * (t - 1) : 50 * t]
        rows_j = np.arange(ROWS * ((m + 4) % 8), ROWS * ((m + 4) % 8) + ROWS)
        feats[rows_j[:, None], kmap[None, :KH]] += fj[:, 150:175]

    out = np.concatenate([x, feats], axis=1)
    if _trace:
        return out, res
    return out


# revision 14
# speedup vs baseline: 1.2495x; 1.0092x over previous
"""Minibatch discrimination kernel v2 — symmetric pair-block schedule, 8 cores.

reference:
    M = (x @ T).reshape(B, K, D)                       # B=1024, K=50, D=5
    abs_diffs[i,k,j] = sum_d |M[i,k,d] - M[j,k,d]|
    feat[i,k] = sum_j exp(-abs_diffs[i,k,j])
    out = concat([x, feat], axis=1)                    # [1024, 562]

v2 exploits |M_i - M_j| symmetry: each unordered block pair {a,b} of the
8x8 (128-row) block grid is computed once. Core m owns query block bi=m
(partitions) vs key blocks bj in {m, m+1, m+2, m+3} (mod 8) for all 50
kernel slots, plus bj=m+4 for slots 0..24 only (its partner core covers
the other 25 via a per-core kernel-slot permutation, keeping the program
SPMD-identical). Row sums of E=exp(-L1) feed feat for bi=m (fused
accum_out); column sums (PE ones-matmuls) feed feat for the bj blocks;
the host adds the partials.

Per-core j axis is rotated by 128*m (host rotates xT per core) so the
owned key blocks are always local columns 0..639. M^T is stored
d-deinterleaved (5 tiles mtd[d][kk, j] = M^T[5*kk+d, j]) so the 5
broadcast matmuls of one kernel slot share a single one-hot lhsT slice
(one weight load per slot instead of five).

Engine split per slot: scalar does Abs planes {0,1}(+{2} on even slots)
fused with the bias subtract + the final Exp (accum_out row-sum); DVE
does plane {2} on odd slots via one tensor_scalar (subtract, abs_max)
plus the 4 fp16 plane adds; gpsimd does planes {3,4} the same one-op way.
"""

import sys

sys.path.insert(0, "/opt/trn_rl_repo")

from contextlib import ExitStack

import numpy as np

import concourse.bass as bass
import concourse.bacc as bacc
import concourse.tile as tile
from concourse import mybir
from concourse.bass_utils import run_bass_kernel_spmd

B, F = 1024, 512
K, D = 50, 5
C = K * D
NCORES = 8
ROWS = B // NCORES  # 128
KH = K // 2  # 25 slots carry the 5th (d4) key block
W_LO, W_HI = 512, 640

f32 = mybir.dt.float32
f16 = mybir.dt.float16

# number of L1-tree adds placed on gpsimd (0 -> all on DVE)
GP_ADDS = 2


def _build_program():
    nc = bacc.Bacc("TRN2", target_bir_lowering=False)

    # Inputs are fp16 (host-converted): halves the load DMAs and runs the
    # M build matmuls at 1 cycle/row instead of fp32's 4.
    # xTrot: x.T with j-columns rotated by 128*m and sliced to the 640
    # local key columns this core ever touches (host-prepared per core)
    xTrot = nc.dram_tensor("xTrot", [F, W_HI], f16, kind="ExternalInput").ap()
    # xTloc: core's own 128 query rows of x, transposed (unrotated)
    xTloc = nc.dram_tensor("xTloc", [F, ROWS], f16, kind="ExternalInput").ap()
    # Tm: T with kernel slots permuted per core (slot s = kernel kern_m(s))
    Tm = nc.dram_tensor("Tm", [F, C], f16, kind="ExternalInput").ap()
    onehot = nc.dram_tensor("onehot", [64, 32 * 128], f16, kind="ExternalInput").ap()
    feat_i = nc.dram_tensor("feat_i", [ROWS, K], f32, kind="ExternalOutput").ap()
    # feat_j partials: cols 0:50 -> block m+1, 50:100 -> m+2, 100:150 -> m+3,
    # 150:175 -> block m+4 (slots 0..24)
    feat_j = nc.dram_tensor("feat_j", [ROWS, 175], f32, kind="ExternalOutput").ap()

    sub = mybir.AluOpType.subtract
    add = mybir.AluOpType.add

    with tile.TileContext(nc) as tc, ExitStack() as ctx:
        const_pool = ctx.enter_context(tc.tile_pool(name="const", bufs=1))
        plane_pool = ctx.enter_context(tc.tile_pool(name="planes", bufs=16))
        tmp_pool = ctx.enter_context(tc.tile_pool(name="tmps", bufs=10))
        scratch_pool = ctx.enter_context(tc.tile_pool(name="scratch", bufs=6))

        # ---- load inputs -------------------------------------------------
        xt_sb = []
        t_sb = []
        xtl_sb = []
        # Input loads round-robin across engine DMA queues — a single
        # queue moves ~26GB/s and serializes the whole setup phase.
        qs = [nc.sync, nc.scalar, nc.gpsimd]
        for fc in range(4):
            t = const_pool.tile([128, W_HI], f16, tag=f"xt{fc}")
            qs[fc % 3].dma_start(out=t[:], in_=xTrot[128 * fc : 128 * (fc + 1), :])
            xt_sb.append(t)
            t2 = const_pool.tile([128, C], f16, tag=f"tm{fc}")
            qs[(fc + 1) % 3].dma_start(out=t2[:], in_=Tm[128 * fc : 128 * (fc + 1), :])
            t_sb.append(t2)
            t3 = const_pool.tile([128, ROWS], f16, tag=f"xtl{fc}")
            qs[(fc + 2) % 3].dma_start(
                out=t3[:], in_=xTloc[128 * fc : 128 * (fc + 1), :]
            )
            xtl_sb.append(t3)
        # one-hot is the single biggest load — split across two queues
        oh_sb = const_pool.tile([64, 32 * 128], f16, tag="onehot")
        nc.scalar.dma_start(out=oh_sb[0:32, :], in_=onehot[0:32, :])
        nc.gpsimd.dma_start(out=oh_sb[32:64, :], in_=onehot[32:64, :])

        # Build-phase PSUM pool is scoped so its banks return to the
        # allocator before the main-loop pools are created.
        with tc.tile_pool(name="bpsum", bufs=2, space="PSUM") as build_psum:
            # One dummy matmul per DMA-queue sem (walrus S3_LW: <=1 sync
            # wait per fused matmul).
            ps_dummy = build_psum.tile([128, 512], f32, tag="bld", name="ps_dummy")
            for dt_tile in xt_sb + t_sb + xtl_sb + [oh_sb]:
                fw = dt_tile.shape[1]
                nc.tensor.matmul(
                    out=ps_dummy[:, 0 : min(512, fw)],
                    lhsT=dt_tile[0:32, 0 : min(128, fw)],
                    rhs=dt_tile[0:32, 0 : min(512, fw)],
                    start=True,
                    stop=True,
                    tile_position=(0, 0),
                )

            # ---- build mtd[d]: [64, 640] f16, row kk = M^T[5*kk+d, jrot] --
            mtd = []
            for d in range(D):
                t = const_pool.tile([64, W_HI], f16, tag=f"mtd{d}", name=f"mtd{d}")
                nc.vector.memset(t[:, :], 0.0)
                mtd.append(t)
            for d in range(D):
                for lo, hi in ((0, 512), (512, W_HI)):
                    ps = build_psum.tile([128, 512], f32, tag="bld")
                    for fc in range(4):
                        nc.tensor.matmul(
                            out=ps[:K, : hi - lo],
                            lhsT=t_sb[fc].rearrange("p (k d) -> p d k", d=D)[:, d, :],
                            rhs=xt_sb[fc][:, lo:hi],
                            start=(fc == 0),
                            stop=(fc == 3),
                        )
                    nc.scalar.copy(mtd[d][:K, lo:hi], ps[:K, : hi - lo])

            # ---- build mloc [128, 250] = M for core's own query rows -----
            mloc = const_pool.tile([128, C], f32, tag="mloc")
            ps = build_psum.tile([128, 512], f32, tag="bld")
            for fc in range(4):
                nc.tensor.matmul(
                    out=ps[:, :C],
                    lhsT=xtl_sb[fc][:],
                    rhs=t_sb[fc][:],
                    start=(fc == 0),
                    stop=(fc == 3),
                )
            nc.scalar.copy(mloc[:], ps[:, :C])

        fj_psum = ctx.enter_context(tc.tile_pool(name="fjpsum", bufs=1, space="PSUM"))
        bc_psum = ctx.enter_context(tc.tile_pool(name="bcpsum", bufs=3, space="PSUM"))
        # feat_j accumulator: per (slot, block) column, drained once at end.
        fj_ps = fj_psum.tile([128, 175], f32, tag="fjacc", name="fj_ps")

        feat_sb = const_pool.tile([128, K], f32, tag="feat")
        ones16 = const_pool.tile([128, 1], f16, tag="ones16")
        nc.vector.memset(ones16[:, :], 1.0)

        # ---- main loop, software-pipelined over kernel slots -------------
        # Stage A (slot ks):   PE broadcasts + scalar Abs planes + DVE pa/pb
        # Stage B (slot ks-1): gp t01/t23 + DVE s1/s2/l1
        # Stage C (slot ks-2): scalar Exp (+accum feat_i)
        # Stage D (slot ks-3): PE feat_j column-sum matmuls
        # Delaying each consumer stage keeps every engine's in-order stream
        # free of head-of-line waits on the current slot's chain.
        def wof(ks):
            return W_HI if ks < KH else W_LO

        st = {}  # ks -> dict(planes, pa, pb, l1, ex)

        for ks in range(K + 3):
            if ks < K:
                W = wof(ks)
                bbase = (ks // 32) * 32
                c0 = ks % 32
                planes = []
                for d in range(D):
                    c = 5 * ks + d
                    ps = bc_psum.tile([128, W_HI], f32, tag="bc")
                    nchunks = (W + 511) // 512
                    for jh in range(nchunks):
                        lo = 512 * jh
                        hi = min(W, lo + 512)
                        nc.tensor.matmul(
                            out=ps[:, lo:hi],
                            lhsT=oh_sb[bbase : bbase + 32, 128 * c0 : 128 * (c0 + 1)],
                            rhs=mtd[d][bbase : bbase + 32, lo:hi],
                            start=True,
                            stop=True,
                            tile_position=(bbase, 0),
                        )
                    if d < 4:
                        pl = plane_pool.tile([128, W_HI], f16, tag="plane")
                        nc.scalar.activation(
                            pl[:, :W],
                            ps[:, :W],
                            mybir.ActivationFunctionType.Abs,
                            bias=mloc[:, c : c + 1],
                            scale=-1.0,
                        )
                        planes.append(pl)
                    else:
                        # plane 4 on DVE: pa = relu(x-m), pb = min(x-m, 0);
                        # |x-m| = pa - pb is folded into the L1 tree.
                        pa = plane_pool.tile([128, W_HI], f16, tag="pa")
                        nc.vector.tensor_scalar(
                            pa[:, :W], ps[:, :W], mloc[:, c : c + 1], 0.0,
                            op0=sub, op1=mybir.AluOpType.max,
                        )
                        pb = plane_pool.tile([128, W_HI], f16, tag="pb")
                        nc.vector.tensor_scalar(
                            pb[:, :W], ps[:, :W], mloc[:, c : c + 1], 0.0,
                            op0=sub, op1=mybir.AluOpType.min,
                        )
                st[ks] = {"planes": planes, "pa": pa, "pb": pb}

            kb = ks - 1
            if 0 <= kb < K:
                W = wof(kb)
                s = st[kb]
                planes, pa, pb = s["planes"], s["pa"], s["pb"]
                eng01 = nc.gpsimd if GP_ADDS >= 1 else nc.vector
                t01 = tmp_pool.tile([128, W_HI], f16, tag="t01")
                eng01.tensor_tensor(
                    out=t01[:, :W], in0=planes[0][:, :W], in1=planes[1][:, :W], op=add
                )
                eng23 = nc.gpsimd if GP_ADDS >= 2 else nc.vector
                t23 = tmp_pool.tile([128, W_HI], f16, tag="t23")
                eng23.tensor_tensor(
                    out=t23[:, :W], in0=planes[2][:, :W], in1=planes[3][:, :W], op=add
                )
                s1 = tmp_pool.tile([128, W_HI], f16, tag="s1")
                nc.vector.tensor_tensor(
                    out=s1[:, :W], in0=pa[:, :W], in1=t01[:, :W], op=add
                )
                s2 = tmp_pool.tile([128, W_HI], f16, tag="s2")
                nc.vector.tensor_tensor(
                    out=s2[:, :W], in0=s1[:, :W], in1=t23[:, :W], op=add
                )
                l1 = tmp_pool.tile([128, W_HI], f16, tag="l1")
                nc.vector.tensor_tensor(
                    out=l1[:, :W], in0=s2[:, :W], in1=pb[:, :W], op=sub
                )
                s["l1"] = l1

            kc = ks - 2
            if 0 <= kc < K:
                W = wof(kc)
                s = st[kc]
                ex = scratch_pool.tile([128, W_HI], f16, tag="ex")
                nc.scalar.activation(
                    ex[:, :W],
                    s["l1"][:, :W],
                    mybir.ActivationFunctionType.Exp,
                    bias=0.0,
                    scale=-1.0,
                    accum_out=feat_sb[:, kc : kc + 1],
                )
                s["ex"] = ex

            kd = ks - 3
            if 0 <= kd < K:
                W = wof(kd)
                ex = st[kd]["ex"]
                # column sums for the non-diagonal key blocks -> feat_j psum
                for t in range(1, 4):
                    nc.tensor.matmul(
                        out=fj_ps[:, 50 * (t - 1) + kd : 50 * (t - 1) + kd + 1],
                        lhsT=ex[:, 128 * t : 128 * (t + 1)],
                        rhs=ones16[:, 0:1],
                        start=True,
                        stop=True,
                    )
                if W == W_HI:
                    nc.tensor.matmul(
                        out=fj_ps[:, 150 + kd : 150 + kd + 1],
                        lhsT=ex[:, 512:640],
                        rhs=ones16[:, 0:1],
                        start=True,
                        stop=True,
                    )
                del st[kd]

        fj_sb = const_pool.tile([128, 175], f32, tag="fjsb")
        nc.vector.tensor_scalar(fj_sb[:], fj_ps[:], 0.0, None, op0=add)
        nc.sync.dma_start(out=feat_i[:, :], in_=feat_sb[:, :K])
        nc.sync.dma_start(out=feat_j[:, :], in_=fj_sb[:, :])

    nc.compile()
    return nc


_program_cache = {}


def _get_program():
    if "nc" not in _program_cache:
        _program_cache["nc"] = _build_program()
    return _program_cache["nc"]


def _make_onehot():
    oh = np.zeros((64, 32 * 128), dtype=np.float16)
    for p in range(64):
        oh[p, (p % 32) * 128 : (p % 32 + 1) * 128] = 1.0
    return oh


def kernel(x: np.ndarray, T: np.ndarray, _trace=False, _trace_kwargs=None):
    x = np.asarray(x, dtype=np.float32)
    T = np.asarray(T, dtype=np.float32)
    nc = _get_program()

    xT_full = np.ascontiguousarray(x.T)  # [512, 1024]
    oh = _make_onehot()
    T5 = T.reshape(F, K, D)
    in_maps = []
    kmaps = []
    for m in range(NCORES):
        # kernel-slot permutation: slots 0..24 must be this core's d4 half
        if m < 4:
            kmap = np.arange(K)
        else:
            kmap = np.concatenate([np.arange(KH, K), np.arange(0, KH)])
        kmaps.append(kmap)
        Tperm = np.ascontiguousarray(T5[:, kmap, :].reshape(F, C)).astype(np.float16)
        in_maps.append(
            {
                "xTrot": np.ascontiguousarray(
                    np.roll(xT_full, -ROWS * m, axis=1)[:, :W_HI]
                ).astype(np.float16),
                "xTloc": np.ascontiguousarray(
                    xT_full[:, ROWS * m : ROWS * (m + 1)]
                ).astype(np.float16),
                "Tm": Tperm,
                "onehot": oh,
            }
        )

    res = run_bass_kernel_spmd(
        nc,
        in_maps,
        core_ids=list(range(NCORES)),
        trace=_trace,
        **(_trace_kwargs or {}),
    )

    feats = np.zeros((B, K), dtype=np.float32)
    for m in range(NCORES):
        kmap = kmaps[m]
        fi = np.asarray(res.results[m]["feat_i"], dtype=np.float32)  # [128, 50]
        fj = np.asarray(res.results[m]["feat_j"], dtype=np.float32)  # [128, 175]
        rows_i = slice(ROWS * m, ROWS * (m + 1))
        # feat_i: row sums over owned key blocks, slot s = kernel kmap[s]
        feats[rows_i][:, kmap] += fi
        # feat_j partials for blocks m+1..m+3 (all slots) and m+4 (slots<25)
        for t in range(1, 4):
            rows_j = np.arange(ROWS * ((m + t) % 8), ROWS * ((m + t) % 8) + ROWS)
            feats[rows_j[:, None], kmap[None, :]] += fj[:, 50 * (t - 1) : 50 * t]
        rows_j = np.arange(ROWS * ((m + 4) % 8), ROWS * ((m + 4) % 8) + ROWS)
        feats[rows_j[:, None], kmap[None, :KH]] += fj[:, 150:175]

    out = np.concatenate([x, feats], axis=1)
    if _trace:
        return out, res
    return out
